# revision 24
# baseline (speedup 1.0000x reference)
"""Trainium2 Bass kernel for nn_BasicCGInducer (CKY inside algorithm for a
categorial-grammar inducer).

Strategy (8 NeuronCores):
  - Data-parallel over sentences: core j handles sentences 4j..4j+3.
  - Emission log-partition (the big [C,V] softmax denominator) is
    tensor-parallel over vocab: each core computes sum_v exp(logits) for a
    4000-column V-shard in bf16, then one AllReduce of [C] partial sums.
  - Everything else (grammar tables, split-MLP, beta1, CKY) is computed
    per-core on its sentence shard in scaled-exp space.

Perf notes vs the first working version (896us -> ~498us):
  - all matmuls run in bf16 (4x PE rate), psum accumulation stays fp32
  - no scalar-engine Ln anywhere: lse/softplus/root-ln use a DVE
    polynomial ln; the CKY per-level rescale uses a power-of-two
    normalizer extracted with integer bit ops.  The scalar engine only
    ever runs {Exp, Relu, Identity, Abs} so its activation table is
    loaded once (the Exp/Ln alternation used to cost ~87us in table
    loads).
  - CKY edge products run in bf16 (DVE 2x mode); the 2592->72 group
    reduce is a bf16 tensor_tensor fold tree (tensor_reduce never gets
    the 2x mode, folds do).
  - power-of-two renormalization runs on even levels only; values drift
    far less than fp32 range allows between renorms.
  - beta1 log values are <= 0 by construction (log-softmax + negative
    split), so the per-pair max shift M1 is identically 0 and is removed.
  - split-MLP head is computed in [cats-on-partitions, 21] layout so the
    softplus/adj math runs as 21-cycle DVE ops, not 2688-cycle
    single-partition ops; the adj row reaches beta1's rhs via a PE
    transpose (a strided-scatter DMA took 12us).
  - chartE is split by level parity so the stage prefetch of iteration
    L+1 is never falsely serialized behind iteration L's writeback
    (coarse-grained DMA dependency tracking); prefetch+writeback run on
    the gpsimd DMA queue, the critical block copy on the sync queue.
  - the CKY chain keeps the vector engine ~90% busy; offloading work to
    gpsimd was tried and reverted (gpsimd elementwise ops share SBUF
    ports with the DVE and slow it ~30%).
"""
import sys
import contextlib

sys.path.insert(0, "/opt/trn_rl_repo")

import numpy as np

import concourse.bass as bass
import concourse.bacc as bacc
import concourse.mybir as mybir
import concourse.tile as tile
from concourse.ap import AP
from concourse import bass_utils

F32 = mybir.dt.float32
BF16 = mybir.dt.bfloat16
I32 = mybir.dt.int32
ALU = mybir.AluOpType
ACTF = mybir.ActivationFunctionType
AXIS = mybir.AxisListType

BF16NP = mybir.dt.np(mybir.dt.bfloat16)

# ---------------------------------------------------------------- constants
P4 = 4          # primitive cats
NF = 36         # non-functor cats
C = 2596        # total cats
CP = 2688       # padded C (21 * 128)
NT = CP // 128  # 21 c-tiles
D = 64
B = 32          # total sentences
NCORES = 8
BLOC = B // NCORES  # 4 sentences per core
V = 32000
BLK = 80        # per-level block stride in chart tensors
NEGB = -1.0e5   # bias for padded vocab columns

LN2 = 0.6931471805599453
# ln(m) ~= sum_k LNB[k] * m^k on [1,2] (no constant term; max err ~1.1e-4)
LNB = [-4.70528660059376, 11.918040257343248, -12.100112712668084,
       6.47839771749535, -1.7936781454842305, 0.20274855065437586]


class Cfg:
    def __init__(self, n=32, v_loc=4000, n_cores=8):
        self.n = n                      # sentence length
        self.v_loc = v_loc              # vocab shard per core
        self.v_pad = ((v_loc + 511) // 512) * 512
        self.n_cores = n_cores
        self.pairs = 4 * n              # (i, b) pairs on partitions


# ------------------------------------------------------------ functor maps
def lf_block_offsets(op):
    """c = off + {A: 4r+a | B: 32r+(a-4) | C: 36(r-4)+a} per derivation of
    the deterministic functor-id tables. op=0 -> l_functors, 1 -> r_functors."""
    return {
        "A": 4 + 16 * op,            # res<4, arg<4 : c = A + 4*res + arg
        "B": 36 + 1280 * op,         # res<4, arg>=4: c = B + 32*res + (arg-4)
        "C": 164 + 1280 * op,        # res>=4      : c = C0 + 36*(res-4) + arg
    }


def check_functor_tables(l_functors, r_functors):
    for op, tab in ((0, l_functors), (1, r_functors)):
        off = lf_block_offsets(op)
        exp = np.zeros((NF, NF), np.int64)  # [arg, res]
        for res in range(NF):
            for arg in range(NF):
                if res < P4 and arg < P4:
                    exp[arg, res] = off["A"] + 4 * res + arg
                elif res < P4:
                    exp[arg, res] = off["B"] + 32 * res + (arg - 4)
                else:
                    exp[arg, res] = off["C"] + 36 * (res - 4) + arg
        assert np.array_equal(np.asarray(tab, np.int64), exp), (
            f"functor table structure mismatch (op={op})")


# ---------------------------------------------------------------- AP helper
def mk(t, parts, off, dims, base_part=0):
    """Raw AP on tile t: partition range [base_part, base_part+parts),
    free offset `off` (elements), extra free dims [[step, count], ...]."""
    w = t.ap[0][0]
    return AP(t.tensor, t.offset + base_part * w + off, [[w, parts]] + dims)


def dve_ln(nc, out, in_, scr_f, scr_i, p, w):
    """out[0:p, 0:w] = ln(in_[0:p, 0:w]) via exponent extraction + a
    degree-6 polynomial on the mantissa, entirely on the vector engine.
    in_ must be positive fp32.  scr_f fp32 / scr_i int32 scratch tiles
    (>= w cols each).  Max abs err ~1.1e-4."""
    e_i = scr_i[0:p, 0:w]
    m_i = scr_i[0:p, w:2 * w]
    ef = scr_f[0:p, 0:w]
    ib = in_.bitcast(I32)
    # raw exponent - 127
    nc.vector.tensor_scalar(e_i, ib, 23, None, op0=ALU.logical_shift_right)
    nc.vector.tensor_scalar(e_i, e_i, 127, None, op0=ALU.subtract)
    nc.vector.tensor_copy(ef, e_i)          # int -> float convert
    # mantissa in [1, 2): keep mantissa bits, force exponent field to 127
    nc.vector.tensor_scalar(m_i, ib, 0x007FFFFF, 0x3F800000,
                            op0=ALU.bitwise_and, op1=ALU.bitwise_or)
    m = m_i.bitcast(F32)
    # Horner in the (p + b) * m form: p = sum_k LNB[k] m^k
    nc.vector.tensor_scalar_mul(out, m, LNB[5])
    for k in (4, 3, 2, 1, 0):
        nc.vector.scalar_tensor_tensor(out, out, LNB[k], m,
                                       op0=ALU.add, op1=ALU.mult)
    # out = ef * ln2 + poly
    nc.vector.scalar_tensor_tensor(out, ef, LN2, out,
                                   op0=ALU.mult, op1=ALU.add)


# ============================================================ device program
def build_program(cfg: Cfg):
    nc = bacc.Bacc("TRN2", target_bir_lowering=False, debug=False,
                   num_devices=cfg.n_cores)
    d = {
        "ntembT": nc.dram_tensor("ntembT", [65, CP], BF16,
                                 kind="ExternalInput"),
        "vocabW": nc.dram_tensor("vocabW", [65, cfg.v_pad], BF16,
                                 kind="ExternalInput"),
        "wordW": nc.dram_tensor("wordW", [66, cfg.pairs], BF16,
                                kind="ExternalInput"),
        "mlpW": nc.dram_tensor("mlpW", [64, 322], BF16, kind="ExternalInput"),
        "mlpB": nc.dram_tensor("mlpB", [64, 8], F32, kind="ExternalInput"),
        "ruleWb": nc.dram_tensor("ruleWb", [36, 144], F32,
                                 kind="ExternalInput"),
        "smallv": nc.dram_tensor("smallv", [1, 16], F32,
                                 kind="ExternalInput"),
        "sW2b": nc.dram_tensor("sW2b", [65, 2], BF16,
                                kind="ExternalInput"),
        "ident": nc.dram_tensor("ident", [128, 128], F32,
                                kind="ExternalInput"),
        "out": nc.dram_tensor("out_nll", [BLOC, 1], F32,
                              kind="ExternalOutput"),
    }
    with tile.TileContext(nc) as tc:
        _trace(tc, cfg, d)
    nc.compile()
    return nc


def _trace(tc, cfg, d):
    nc = tc.nc
    n, PAIRS, VP = cfg.n, cfg.pairs, cfg.v_pad
    NV = VP // 512                    # 512-col v-tiles per core
    NHALF = (NV + 2) // 3             # ACT chunks of up to 3 v-tiles

    es = contextlib.ExitStack()
    keep = es.enter_context(tc.tile_pool(name="keep", bufs=1))
    dram = es.enter_context(tc.tile_pool(name="dram", bufs=1, space="DRAM"))

    # ---------------- long-lived tensors
    chartA = keep.tile([PAIRS, (n + 1) * BLK], F32)
    chartEv = keep.tile([PAIRS, (n + 1) * BLK], F32)
    chartEo = keep.tile([PAIRS, (n + 1) * BLK], F32)
    WA = keep.tile([PAIRS, 1312], BF16)
    WB = keep.tile([PAIRS, 1312], BF16)
    glR = keep.tile([128, 1296], F32)
    grR = keep.tile([128, 1296], F32)
    mlpB = keep.tile([64, 8], F32)
    smallv = keep.tile([1, 16], F32)
    sumexp_parts = keep.tile([128, NT * NHALF], F32)
    sumexp_loc = keep.tile([128, NT], F32)
    sumexp_g = keep.tile([128, NT], F32)
    lse21 = keep.tile([128, NT], F32)
    sp21 = keep.tile([128, NT], F32)
    adj21 = keep.tile([128, NT], F32)
    ident = keep.tile([128, 128], F32)
    adjT = keep.tile([NT, 128], BF16)
    lnf = keep.tile([128, 24], F32)          # dve_ln scratch
    lni = keep.tile([128, 48], I32)
    s0E = keep.tile([1, 40], F32)
    rsRep = keep.tile([4, 4], F32)
    fin = keep.tile([4, 16], F32)

    nc.sync.dma_start(mlpB[:], d["mlpB"][:])
    nc.sync.dma_start(smallv[:], d["smallv"][:])
    nc.sync.dma_start(ident[:], d["ident"][:])
    nc.gpsimd.memset(chartA[:], 0.0)
    nc.gpsimd.memset(chartEv[:], 0.0)
    nc.gpsimd.memset(chartEo[:], 0.0)

    ph1 = contextlib.ExitStack()
    p1 = ph1.enter_context(tc.tile_pool(name="ph1", bufs=1))
    ntembT = p1.tile([65, CP], BF16)
    vocabW = p1.tile([65, VP], BF16)
    wordW = p1.tile([66, PAIRS], BF16)
    mlpW = p1.tile([64, 322], BF16)
    ruleWb = p1.tile([36, 144], F32)
    rhs_b = p1.tile([66, CP], BF16)
    beta1E = p1.tile([PAIRS, CP], F32)
    ruleflat = p1.tile([1, 36 * 72], F32)
    hA = p1.tile([65, CP], BF16)
    sW2b = p1.tile([65, 2], BF16)
    hB = p1.tile([64, CP], BF16)
    hC = p1.tile([64, CP], BF16)
    y21 = p1.tile([128, NT], F32)
    u21 = p1.tile([128, NT], F32)
    s0col = p1.tile([128, 1], F32)

    nc.sync.dma_start(ntembT[:], d["ntembT"][:])
    nc.sync.dma_start(vocabW[:], d["vocabW"][:])
    nc.sync.dma_start(wordW[:], d["wordW"][:])
    nc.sync.dma_start(mlpW[:], d["mlpW"][:])
    nc.sync.dma_start(sW2b[:], d["sW2b"][:])
    nc.gpsimd.memset(hA[64:65, :], 1.0)
    nc.sync.dma_start(ruleWb[:], d["ruleWb"][:])
    nc.sync.dma_start(rhs_b[0:65, :], ntembT[:])

    # =======================================================================
    # Phase 1: emission partition function (bf16 matmul, exp+accum on scalar)
    # =======================================================================
    with tc.tile_pool(name="psum_e", bufs=2, space="PSUM") as pse, \
         tc.tile_pool(name="scr_e", bufs=2) as scre:
        for ct in range(NT):
            for h in range(NHALF):
                vt0 = h * 3
                nvt = min(3, NV - vt0)
                pt = pse.tile([128, 512 * nvt], F32, tag="pse")
                for vt in range(nvt):
                    nc.tensor.matmul(
                        pt[:, vt * 512:(vt + 1) * 512],
                        ntembT[:, ct * 128:(ct + 1) * 128],
                        vocabW[:, (vt0 + vt) * 512:(vt0 + vt + 1) * 512],
                        start=True, stop=True)
                sce = scre.tile([128, 512 * 3], F32, tag="scre")
                nc.scalar.activation(
                    sce[:, 0:512 * nvt], pt[:], ACTF.Exp,
                    accum_out=sumexp_parts[:, ct * NHALF + h:
                                           ct * NHALF + h + 1])

    if NHALF > 1:
        nc.vector.tensor_reduce(
            sumexp_loc[:],
            mk(sumexp_parts, 128, 0, [[NHALF, NT], [1, NHALF]]),
            axis=AXIS.X, op=ALU.add)
    else:
        nc.vector.tensor_copy(sumexp_loc[:], sumexp_parts[:, 0:NT])

    # AllReduce over cores via DRAM bounce
    cc_in = dram.tile([128, NT], F32)
    cc_out = dram.tile([128, NT], F32)
    nc.sync.dma_start(cc_in[:], sumexp_loc[:])
    nc.gpsimd.collective_compute(
        "AllReduce", ALU.add,
        replica_groups=[list(range(cfg.n_cores))],
        ins=[cc_in[:].opt()], outs=[cc_out[:].opt()])
    nc.sync.dma_start(sumexp_g[:], cc_out[:])

    # =======================================================================
    # Phase 2: split MLP (transposed layout hT [64, CP]), rule tables, root
    # (emitted after emission so the tensor queue drains emission first)
    # =======================================================================
    psm = ph1.enter_context(tc.tile_pool(name="psum_m", bufs=2, space="PSUM"))
    scrs = ph1.enter_context(tc.tile_pool(name="scr_s", bufs=2))
    hp = tc.high_priority

    def dense_relu(dst, col0, rhs, bias_col, res_add=None, func=ACTF.Relu):
        with hp():
            for c0 in range(0, CP, 512):
                c1 = min(c0 + 512, CP)
                pm = psm.tile([128, 512], F32, tag="psm")
                nc.tensor.matmul(pm[0:64, 0:c1 - c0],
                                 mlpW[:, col0:col0 + 64],
                                 rhs[0:64, c0:c1],
                                 start=True, stop=True)
                nc.scalar.activation(
                    dst[0:64, c0:c1], pm[0:64, 0:c1 - c0], func,
                    bias=mlpB[:, bias_col:bias_col + 1])
                if res_add is not None:
                    nc.vector.tensor_tensor(
                        dst[0:64, c0:c1], dst[0:64, c0:c1],
                        res_add[0:64, c0:c1], op=ALU.add)

    dense_relu(hA, 0, ntembT, 0, func=ACTF.Identity)   # h1 (linear)
    dense_relu(hB, 64, hA, 1)                           # t = relu(h1 W + b)
    dense_relu(hC, 128, hB, 2, res_add=hA)              # h2
    dense_relu(hB, 192, hC, 3)                          # t2
    dense_relu(hA, 256, hB, 4, res_add=hC)              # h3 (rows 0:64)

    # split head, transposed: per c-tile psum [128, 2] incl sb2 bias row;
    # y = s0 - s1
    with hp():
        for ct in range(NT):
            ps = psm.tile([128, 512], F32, tag="psm")
            nc.tensor.matmul(ps[0:128, 0:2], hA[:, ct * 128:(ct + 1) * 128],
                             sW2b[:], start=True, stop=True)
            y2t = scrs.tile([128, 2], F32, tag="y2t")
            nc.vector.tensor_copy(y2t[:], ps[0:128, 0:2])
            nc.vector.tensor_tensor(
                y21[:, ct:ct + 1], y2t[:, 0:1], y2t[:, 1:2], op=ALU.subtract)

    # softplus(y) = max(y,0) + ln(1 + exp(-|y|)) with DVE ln
    with hp():
        nc.vector.tensor_scalar(u21[:].bitcast(I32), y21[:].bitcast(I32),
                                0x7FFFFFFF, None, op0=ALU.bitwise_and)
        nc.scalar.activation(u21[:], u21[:], ACTF.Exp, scale=-1.0)
        nc.vector.tensor_scalar_add(u21[:], u21[:], 1.0)
        dve_ln(nc, u21[:], u21[:], lnf, lni, 128, NT)   # ln(1+e^-|y|)
        nc.vector.tensor_scalar_max(sp21[:], y21[:], 0.0)
        nc.vector.tensor_tensor(sp21[:], sp21[:], u21[:], op=ALU.add)

        # split0 = y - softplus(y); s0E = exp(split0) for cats 0..35 as a row
        nc.vector.tensor_tensor(s0col[:], y21[:, 0:1], sp21[:, 0:1],
                                op=ALU.subtract)
        nc.scalar.activation(s0col[:], s0col[:], ACTF.Exp)
        s0_d = dram.tile([NF, 1], F32)
        nc.sync.dma_start(s0_d[:], s0col[0:NF, :])
        nc.sync.dma_start(
            AP(s0E.tensor, s0E.offset, [[s0E.ap[0][0], 1], [1, NF]]),
            AP(s0_d.tensor, s0_d.offset, [[s0_d.ap[0][0], 1], [1, NF]]))

    # rule tables: softmax over 72 per res row
    hp_rules = hp()
    hp_rules.__enter__()
    rsum = keep.tile([36, 72], F32)
    rmax = keep.tile([36, 2], F32)
    rsumexp = keep.tile([36, 2], F32)
    nc.vector.tensor_tensor(rsum[:], ruleWb[:, 0:72], ruleWb[:, 72:144],
                            op=ALU.add)
    nc.vector.tensor_reduce(rmax[:, 0:1], rsum[:], axis=AXIS.X, op=ALU.max)
    nc.vector.tensor_scalar_mul(rmax[:, 1:2], rmax[:, 0:1], -1.0)
    nc.scalar.activation(rsum[:], rsum[:], ACTF.Exp, bias=rmax[:, 1:2],
                         accum_out=rsumexp[:, 0:1])
    nc.vector.reciprocal(rsumexp[:, 1:2], rsumexp[:, 0:1])
    nc.vector.tensor_scalar_mul(rsum[:], rsum[:], rsumexp[:, 1:2])

    # flatten ruleEn to [1, 2592] via DRAM, then G-flats replicated
    rule_d = dram.tile([36, 72], F32)
    nc.sync.dma_start(rule_d[:], rsum[:])
    nc.sync.dma_start(
        AP(ruleflat.tensor, ruleflat.offset,
           [[ruleflat.ap[0][0], 1], [1, 36 * 72]]),
        rule_d[:])
    g_d = dram.tile([2, 1296], F32)
    gtmp = keep.tile([1, 1296], F32)
    for row, off in ((0, 0), (1, 36)):   # 0: Gl (larg), 1: Gr (rarg)
        nc.vector.tensor_tensor(
            gtmp[:],
            mk(ruleflat, 1, off, [[72, 36], [1, 36]]),
            mk(s0E, 1, 0, [[1, 36], [0, 36]]),
            op=ALU.mult)
        nc.sync.dma_start(g_d[row:row + 1, :], gtmp[:])
    for dstt, row in ((glR, 0), (grR, 1)):
        nc.sync.dma_start(
            dstt[:],
            AP(g_d.tensor, g_d.offset + row * g_d.ap[0][0],
               [[0, 128], [1, 1296]]))

    # root: rsEn = softmax(root_W[0,0:4] + root_b[0:4]) replicated to 4 parts
    rs4 = keep.tile([1, 8], F32)
    rsE = keep.tile([1, 8], F32)
    nc.vector.tensor_tensor(rs4[:, 0:4], smallv[:, 2:6], smallv[:, 6:10],
                            op=ALU.add)
    nc.vector.tensor_reduce(rs4[:, 4:5], rs4[:, 0:4], axis=AXIS.X, op=ALU.max)
    nc.vector.tensor_scalar_mul(rs4[:, 5:6], rs4[:, 4:5], -1.0)
    nc.scalar.activation(rsE[:, 0:4], rs4[:, 0:4], ACTF.Exp,
                         bias=rs4[:, 5:6], accum_out=rsE[:, 4:5])
    nc.vector.reciprocal(rsE[:, 5:6], rsE[:, 4:5])
    nc.vector.tensor_scalar_mul(rsE[:, 0:4], rsE[:, 0:4], rsE[:, 5:6])
    rs_d = dram.tile([1, 4], F32)
    nc.sync.dma_start(rs_d[:], rsE[:, 0:4])
    nc.sync.dma_start(rsRep[:],
                      AP(rs_d.tensor, rs_d.offset, [[0, 4], [1, 4]]))
    hp_rules.__exit__(None, None, None)

    # =======================================================================
    # Phase 3: adj -> beta1 logits -> exp tables WA/WB (M1 == 0: beta1 log
    # values are <= 0 because dist is a log-softmax and split1 <= 0)
    # =======================================================================
    # adj = -softplus(y) - lse, in [128, NT] layout; PE-transpose to a row
    with hp():
        dve_ln(nc, lse21[:], sumexp_g[:], lnf, lni, 128, NT)
        nc.vector.scalar_tensor_tensor(adj21[:], sp21[:], -1.0, lse21[:],
                                       op0=ALU.mult, op1=ALU.subtract)
        with tc.tile_pool(name="psum_t", bufs=1, space="PSUM") as pst:
            pT = pst.tile([NT, 128], F32)
            nc.tensor.transpose(pT[:], adj21[:], ident[:])
            nc.vector.tensor_copy(adjT[:], pT[:])   # fp32 psum -> bf16 sbuf
        nc.sync.dma_start(
            AP(rhs_b.tensor, rhs_b.offset + 65 * rhs_b.ap[0][0],
               [[rhs_b.ap[0][0], 1], [128, NT], [1, 128]]),
            adjT[:])

    with tc.tile_pool(name="psum_b", bufs=1, space="PSUM") as psb:
        pb = psb.tile([PAIRS, CP], F32)
        for c0 in range(0, CP, 512):
            c1 = min(c0 + 512, CP)
            nc.tensor.matmul(pb[:, c0:c1], wordW[:], rhs_b[:, c0:c1],
                             start=True, stop=True)
        nc.scalar.activation(beta1E[:], pb[:], ACTF.Exp)

    # W tables [PAIRS, 1296] bf16: WB = gather_lf(beta1E)*GrE, WA = rf/GlE
    for W, op_id, gR in ((WB, 0, grR), (WA, 1, glR)):
        off = lf_block_offsets(op_id)
        blocks = [
            (0, [[36, 4], [1, 4]], off["A"], [[4, 4], [1, 4]]),
            (4, [[36, 4], [1, 32]], off["B"], [[32, 4], [1, 32]]),
            (144, [[1, 1152]], off["C"], [[1, 1152]]),
        ]
        for (oo, od, io, idm) in blocks:
            nc.vector.scalar_tensor_tensor(
                mk(W, PAIRS, oo, od),
                mk(beta1E, PAIRS, io, idm),
                1.0,
                mk(gR, PAIRS, oo, od),
                op0=ALU.mult, op1=ALU.mult)

    # chart block L=1 from beta1E (scale col stays 0 from the memset)
    nc.vector.tensor_copy(mk(chartA, PAIRS, BLK, [[1, 36]]), beta1E[:, 0:NF])
    nc.vector.tensor_tensor(mk(chartA, PAIRS, BLK + 40, [[4, 4], [1, 4]]),
                            mk(beta1E, PAIRS, 20, [[4, 4], [1, 4]]),
                            mk(glR, PAIRS, 0, [[36, 4], [1, 4]]),
                            op=ALU.mult)
    nc.vector.tensor_tensor(mk(chartA, PAIRS, BLK + 56, [[4, 4], [1, 4]]),
                            mk(beta1E, PAIRS, 4, [[4, 4], [1, 4]]),
                            mk(grR, PAIRS, 0, [[36, 4], [1, 4]]),
                            op=ALU.mult)
    # chartEnd block m lives at col (n-m)*BLK (reversed layout; makes all
    # k-strided reads positive-step). Block 1: end j = i+1 -> row pair.
    nc.sync.dma_start(mk(chartEo, PAIRS, (n - 1) * BLK, [[1, BLK]]),
                      mk(chartA, PAIRS, BLK, [[1, BLK]]))

    ph1.close()  # free ph1 tensors before the CKY working set

    es2 = contextlib.ExitStack()
    stage_pool = es2.enter_context(tc.tile_pool(name="stage", bufs=2))
    wash_pool = es2.enter_context(tc.tile_pool(name="wash", bufs=2))
    scr = es2.enter_context(tc.tile_pool(name="cky", bufs=2))
    scr1 = es2.enter_context(tc.tile_pool(name="cky1", bufs=1))

    # =======================================================================
    # Phase 4: CKY in scaled-exp space
    # chartA[pair, L*BLK+.]: 0:36 values | 36 scale | 40:56 FA | 56:72 FB
    # chartE[(j-1)*4+b, ...] same, indexed by span end j.
    # =======================================================================
    for L in range(2, n + 1):
        S = n - L + 1
        PS = 4 * S
        NI = L - 2

        stageE = stage_pool.tile([128, n * BLK], F32, tag="st")
        # critical part first on the sync queue: block L-1 rows [4 .. 4+PS]
        nc.sync.dma_start(
            mk(stageE, PS, (n - L + 1) * BLK, [[1, BLK]]),
            mk(chartA, PS, (L - 1) * BLK, [[1, BLK]], base_part=4))
        # prefetchable part: blocks 1..L-2 on the scalar queue, split by the
        # parity of the writing iteration so a fresh writeback never blocks it
        for par, chE in ((0, chartEv), (1, chartEo)):
            ms = [m for m in range(1, L - 1) if m % 2 == par]
            if not ms:
                continue
            mmax = max(ms)
            nc.gpsimd.dma_start(
                mk(stageE, PS, (n - mmax) * BLK,
                   [[2 * BLK, len(ms)], [1, BLK]]),
                mk(chE, PS, (n - mmax) * BLK,
                   [[2 * BLK, len(ms)], [1, BLK]], base_part=4 * (L - 1)))

        wash = wash_pool.tile([128, 1312], BF16, tag="wa")
        nc.gpsimd.dma_start(
            mk(wash, PS, 0, [[1, 1296]]),
            mk(WA, PS, 0, [[1, 1296]], base_part=4 * (L - 1)))

        # ---- scales: sAsm = [sB | sA | sI(k=1..L-1)]
        sAsm = scr.tile([128, n + 8], F32, tag="sasm")
        nc.vector.tensor_copy(
            sAsm[0:PS, 0:1],
            mk(stageE, PS, (n - L + 1) * BLK + 36, [[1, 1]]))
        nc.vector.tensor_copy(
            sAsm[0:PS, 1:2],
            mk(chartA, PS, (L - 1) * BLK + 36, [[1, 1]]))
        nc.vector.tensor_tensor(
            sAsm[0:PS, 2:L + 1],
            mk(chartA, PS, BLK + 36, [[BLK, L - 1]]),
            mk(stageE, PS, (n - L + 1) * BLK + 36, [[BLK, L - 1]]),
            op=ALU.add)
        mstar = scr.tile([128, 2], F32, tag="mstar")
        nc.vector.tensor_reduce(mstar[0:PS, 0:1], sAsm[0:PS, 0:L + 1],
                                axis=AXIS.X, op=ALU.max)
        nc.vector.tensor_scalar_mul(mstar[0:PS, 1:2], mstar[0:PS, 0:1], -1.0)
        eAll = scr.tile([128, n + 8], F32, tag="eall")
        nc.scalar.activation(eAll[0:PS, 0:L + 1], sAsm[0:PS, 0:L + 1],
                             ACTF.Exp, bias=mstar[0:PS, 1:2])

        # ---- edge products in bf16 (DVE 2x): prodAB [PS, 2592]
        ea36 = scr.tile([128, 72], BF16, tag="ea36")
        nc.vector.tensor_scalar_mul(
            ea36[0:PS, 0:36],
            mk(chartA, PS, (L - 1) * BLK, [[1, 36]]), eAll[0:PS, 1:2])
        nc.vector.tensor_scalar_mul(
            ea36[0:PS, 36:72],
            mk(stageE, PS, (n - L + 1) * BLK, [[1, 36]]), eAll[0:PS, 0:1])
        prodAB = scr1.tile([128, 2592], BF16, tag="prod")
        nc.vector.tensor_tensor(
            mk(prodAB, PS, 0, [[1, 1296]]),
            mk(wash, PS, 0, [[1, 1296]]),
            mk(ea36, PS, 0, [[0, 36], [1, 36]]),
            op=ALU.mult)
        nc.vector.tensor_tensor(
            mk(prodAB, PS, 1296, [[1, 1296]]),
            mk(WB, PS, 0, [[1, 1296]]),
            mk(ea36, PS, 36, [[0, 36], [1, 36]]),
            op=ALU.mult)
        redT = scr.tile([128, 1296], BF16, tag="redt")
        nc.vector.tensor_tensor(          # fold 36 -> 18 (bf16, 4x)
            mk(redT, PS, 0, [[18, 72], [1, 18]]),
            mk(prodAB, PS, 0, [[36, 72], [1, 18]]),
            mk(prodAB, PS, 18, [[36, 72], [1, 18]]), op=ALU.add)
        red9 = scr.tile([128, 648], BF16, tag="red9")
        nc.vector.tensor_tensor(          # fold 18 -> 9 (bf16, 4x)
            mk(red9, PS, 0, [[9, 72], [1, 9]]),
            mk(redT, PS, 0, [[18, 72], [1, 9]]),
            mk(redT, PS, 9, [[18, 72], [1, 9]]), op=ALU.add)
        red72 = scr.tile([128, 72], BF16, tag="red")
        with nc.allow_low_precision("bf16 CKY group sums, tol 2e-2"):
            nc.vector.tensor_reduce(red72[0:PS, :],
                                    mk(red9, PS, 0, [[9, 72], [1, 9]]),
                                    axis=AXIS.X, op=ALU.add)
        total36 = scr.tile([128, 40], F32, tag="tot")
        tdst = (total36[0:PS, 0:36] if L % 2 == 0
                else mk(chartA, PS, L * BLK, [[1, 36]]))
        nc.vector.tensor_tensor(tdst, red72[0:PS, 0:36],
                                red72[0:PS, 36:72], op=ALU.add)

        # ---- interior terms (res<4), batched over k
        if NI > 0:
            tI = scr1.tile([128, 2 * max(n - 2, 1) * 16], F32, tag="ti")
            nc.vector.tensor_tensor(   # IA: chart[k][i] args x stage FA(L-k)
                mk(tI, PS, 0, [[2 * NI * 4, 4], [4, NI], [1, 4]]),
                mk(chartA, PS, BLK, [[0, 4], [BLK, NI], [1, 4]]),
                mk(stageE, PS, (n - L + 1) * BLK + 40,
                   [[4, 4], [BLK, NI], [1, 4]]),
                op=ALU.mult)
            nc.vector.tensor_tensor(   # IB: stage args(L-k) x chart[k] FB
                mk(tI, PS, NI * 4, [[2 * NI * 4, 4], [4, NI], [1, 4]]),
                mk(stageE, PS, (n - L + 2) * BLK, [[0, 4], [BLK, NI], [1, 4]]),
                mk(chartA, PS, 2 * BLK + 56, [[4, 4], [BLK, NI], [1, 4]]),
                op=ALU.mult)
            for half in range(2):      # x eI (k scales), in place
                nc.vector.tensor_tensor(
                    mk(tI, PS, half * NI * 4,
                       [[2 * NI * 4, 4], [4, NI], [1, 4]]),
                    mk(tI, PS, half * NI * 4,
                       [[2 * NI * 4, 4], [4, NI], [1, 4]]),
                    mk(eAll, PS, 2 + half, [[0, 4], [1, NI], [0, 4]]),
                    op=ALU.mult)
            nc.vector.tensor_reduce(   # sum over (side*k, arg) -> [PS, 4]
                total36[0:PS, 36:40],
                mk(tI, PS, 0,
                   [[2 * NI * 4, 4], [4, 2 * NI], [1, 4]]),
                axis=AXIS.XY, op=ALU.add)
            idst = (total36[0:PS, 0:4] if L % 2 == 0
                    else mk(chartA, PS, L * BLK, [[1, 4]]))
            nc.vector.tensor_tensor(idst, idst,
                                    total36[0:PS, 36:40], op=ALU.add)

        # ---- rescale by a power of two (integer bit ops; no Ln, no recip).
        # Values drift at most ~e^-40/level, so renormalizing every other
        # level keeps everything in fp32 range at half the bookkeeping.
        if L % 2 == 0:
            mval = scr.tile([128, 2], F32, tag="mval")
            mvi = scr.tile([128, 4], I32, tag="mvi")
            nc.vector.tensor_reduce(mval[0:PS, 0:1], total36[0:PS, 0:36],
                                    axis=AXIS.X, op=ALU.max)
            nc.vector.tensor_scalar(          # t = raw_exponent - 127
                mvi[0:PS, 0:1], mval[0:PS, 0:1].bitcast(I32), 23, None,
                op0=ALU.logical_shift_right)
            nc.vector.tensor_scalar(
                mvi[0:PS, 0:1], mvi[0:PS, 0:1], 127, None, op0=ALU.subtract)
            nc.vector.tensor_copy(mval[0:PS, 1:2], mvi[0:PS, 0:1])  # float(t)
            nc.vector.scalar_tensor_tensor(   # scale_L = t*ln2 + mstar
                mk(chartA, PS, L * BLK + 36, [[1, 1]]),
                mval[0:PS, 1:2], LN2, mstar[0:PS, 0:1],
                op0=ALU.mult, op1=ALU.add)
            nc.vector.tensor_scalar(          # recip bits = (127 - t) << 23
                mvi[0:PS, 1:2], mvi[0:PS, 0:1], -1, 127,
                op0=ALU.mult, op1=ALU.add)
            nc.vector.tensor_scalar(
                mvi[0:PS, 1:2], mvi[0:PS, 1:2], 23, None,
                op0=ALU.logical_shift_left)
            nc.vector.tensor_scalar_mul(
                mk(chartA, PS, L * BLK, [[1, 36]]),
                total36[0:PS, 0:36], mvi[0:PS, 1:2].bitcast(F32))
        else:
            nc.vector.tensor_copy(mk(chartA, PS, L * BLK + 36, [[1, 1]]),
                                  mstar[0:PS, 0:1])
        nc.vector.tensor_tensor(
            mk(chartA, PS, L * BLK + 40, [[4, 4], [1, 4]]),
            mk(chartA, PS, L * BLK + 20, [[4, 4], [1, 4]]),
            mk(glR, PS, 0, [[36, 4], [1, 4]]), op=ALU.mult)
        nc.vector.tensor_tensor(
            mk(chartA, PS, L * BLK + 56, [[4, 4], [1, 4]]),
            mk(chartA, PS, L * BLK + 4, [[4, 4], [1, 4]]),
            mk(grR, PS, 0, [[36, 4], [1, 4]]), op=ALU.mult)
        if L < n:   # chartEnd block L at rows (i+L-1)*4+b, col (n-L)*BLK
            nc.gpsimd.dma_start(
                mk(chartEo if L % 2 else chartEv, PS, (n - L) * BLK,
                   [[1, BLK]], base_part=4 * (L - 1)),
                mk(chartA, PS, L * BLK, [[1, BLK]]))

    # =======================================================================
    # Phase 5: root -> nll per sentence
    # =======================================================================
    nc.vector.tensor_tensor(fin[:, 0:4],
                            mk(chartA, 4, n * BLK, [[1, 4]]),
                            rsRep[:], op=ALU.mult)
    nc.vector.tensor_reduce(fin[:, 4:5], fin[:, 0:4], axis=AXIS.X, op=ALU.add)
    dve_ln(nc, fin[:, 5:6], fin[:, 4:5], lnf, lni, 4, 1)
    nc.vector.scalar_tensor_tensor(
        fin[:, 6:7], fin[:, 5:6], -1.0,
        mk(chartA, 4, n * BLK + 36, [[1, 1]]),
        op0=ALU.mult, op1=ALU.subtract)
    nc.sync.dma_start(d["out"][:], fin[:, 6:7])
    es2.close()
    es.close()


# ============================================================== host wrapper
_PROG_CACHE = {}


def _get_program(cfg: Cfg):
    key = (cfg.n, cfg.v_loc, cfg.n_cores)
    if key not in _PROG_CACHE:
        _PROG_CACHE[key] = build_program(cfg)
    return _PROG_CACHE[key]


def make_inmaps(cfg: Cfg, inputs):
    """Host-side shard/pack of FULL inputs -> per-core DRAM input dicts."""
    x = np.asarray(inputs["x"])
    check_functor_tables(np.asarray(inputs["l_functors"]),
                         np.asarray(inputs["r_functors"]))
    nt_emb = np.asarray(inputs["nt_emb"], np.float32)          # [C, D]
    vocab_W = np.asarray(inputs["vocab_W"], np.float32)        # [D, V]
    vocab_b = np.asarray(inputs["vocab_b"], np.float32)        # [V]

    ntembT = np.zeros((65, CP), np.float32)
    ntembT[0:64, 0:C] = nt_emb.T
    ntembT[64, :] = 1.0
    ntembT = ntembT.astype(BF16NP)

    mlpW = np.zeros((64, 322), np.float32)
    for j, k in enumerate(("sW1", "r1W1", "r1W2", "r2W1", "r2W2")):
        mlpW[:, j * 64:(j + 1) * 64] = np.asarray(inputs[k], np.float32)
    mlpW[:, 320:322] = np.asarray(inputs["sW2"], np.float32)
    mlpW = mlpW.astype(BF16NP)

    mlpB = np.zeros((64, 8), np.float32)
    for j, k in enumerate(("sb1", "r1b1", "r1b2", "r2b1", "r2b2")):
        mlpB[:, j] = np.asarray(inputs[k], np.float32)

    ruleWb = np.zeros((36, 144), np.float32)
    ruleWb[:, 0:72] = np.asarray(inputs["rule_W"], np.float32)
    ruleWb[:, 72:144] = np.tile(
        np.asarray(inputs["rule_b"], np.float32)[None, :], (36, 1))

    smallv = np.zeros((1, 16), np.float32)
    smallv[0, 0:2] = np.asarray(inputs["sb2"], np.float32)
    smallv[0, 2:6] = np.asarray(inputs["root_W"], np.float32)[0, 0:4]
    smallv[0, 6:10] = np.asarray(inputs["root_b"], np.float32)[0:4]

    ident_np = np.eye(128, dtype=np.float32)
    sW2b = np.zeros((65, 2), np.float32)
    sW2b[0:64, :] = np.asarray(inputs["sW2"], np.float32)
    sW2b[64, :] = np.asarray(inputs["sb2"], np.float32)
    sW2b = sW2b.astype(BF16NP)

    vs = cfg.v_loc
    in_maps = []
    for core in range(cfg.n_cores):
        vocabW = np.zeros((65, cfg.v_pad), np.float32)
        vocabW[64, :] = NEGB
        vocabW[0:64, 0:vs] = vocab_W[:, core * vs:(core + 1) * vs]
        vocabW[64, 0:vs] = vocab_b[core * vs:(core + 1) * vs]

        words = x[core * BLOC:(core + 1) * BLOC, 0:cfg.n]   # [BLOC, n]
        wid = words.T.reshape(-1)                           # pair = i*4 + b
        wordW = np.zeros((66, cfg.pairs), np.float32)
        wordW[0:64, :] = vocab_W[:, wid]
        wordW[64, :] = vocab_b[wid]
        wordW[65, :] = 1.0

        in_maps.append({
            "ntembT": ntembT, "vocabW": vocabW.astype(BF16NP),
            "wordW": wordW.astype(BF16NP),
            "mlpW": mlpW, "mlpB": mlpB, "ruleWb": ruleWb, "smallv": smallv,
            "sW2b": sW2b, "ident": ident_np,
        })
    return in_maps


def kernel(**inputs) -> np.ndarray:
    cfg = Cfg(n=32, v_loc=V // NCORES, n_cores=NCORES)
    nc = _get_program(cfg)
    in_maps = make_inmaps(cfg, inputs)
    res = bass_utils.run_bass_kernel_spmd(
        nc, in_maps, core_ids=list(range(cfg.n_cores)))
    out = np.concatenate([r["out_nll"].reshape(-1) for r in res.results])
    return out.astype(np.float32)


if __name__ == "__main__":
    from reference import setup_inputs, reference
    inputs = {k: np.asarray(v) for k, v in setup_inputs().items()}
    got = kernel(**inputs)
    exp = np.asarray(reference(**inputs))
    rel = np.max(np.abs(got - exp) / np.maximum(np.abs(exp), 1e-6))
    print("expected:", exp[:8])
    print("got     :", got[:8])
    print("Relative error:", rel)


# revision 25
# speedup vs baseline: 1.0028x; 1.0028x over previous
"""Trainium2 Bass kernel for nn_BasicCGInducer (CKY inside algorithm for a
categorial-grammar inducer).

Strategy (8 NeuronCores):
  - Data-parallel over sentences: core j handles sentences 4j..4j+3.
  - Emission log-partition (the big [C,V] softmax denominator) is
    tensor-parallel over vocab: each core computes sum_v exp(logits) for a
    4000-column V-shard in bf16, then one AllReduce of [C] partial sums.
  - Everything else (grammar tables, split-MLP, beta1, CKY) is computed
    per-core on its sentence shard in scaled-exp space.

Perf notes vs the first working version (896us -> ~498us):
  - all matmuls run in bf16 (4x PE rate), psum accumulation stays fp32
  - no scalar-engine Ln anywhere: lse/softplus/root-ln use a DVE
    polynomial ln; the CKY per-level rescale uses a power-of-two
    normalizer extracted with integer bit ops.  The scalar engine only
    ever runs {Exp, Relu, Identity, Abs} so its activation table is
    loaded once (the Exp/Ln alternation used to cost ~87us in table
    loads).
  - CKY edge products run in bf16 (DVE 2x mode); the 2592->72 group
    reduce is a bf16 tensor_tensor fold tree (tensor_reduce never gets
    the 2x mode, folds do).
  - power-of-two renormalization runs on even levels only; values drift
    far less than fp32 range allows between renorms.
  - beta1 log values are <= 0 by construction (log-softmax + negative
    split), so the per-pair max shift M1 is identically 0 and is removed.
  - split-MLP head is computed in [cats-on-partitions, 21] layout so the
    softplus/adj math runs as 21-cycle DVE ops, not 2688-cycle
    single-partition ops; the adj row reaches beta1's rhs via a PE
    transpose (a strided-scatter DMA took 12us).
  - chartE is split by level parity so the stage prefetch of iteration
    L+1 is never falsely serialized behind iteration L's writeback
    (coarse-grained DMA dependency tracking); prefetch+writeback run on
    the gpsimd DMA queue, the critical block copy on the sync queue.
  - the CKY chain keeps the vector engine ~90% busy; offloading work to
    gpsimd was tried and reverted (gpsimd elementwise ops share SBUF
    ports with the DVE and slow it ~30%).
"""
import sys
import contextlib

sys.path.insert(0, "/opt/trn_rl_repo")

import numpy as np

import concourse.bass as bass
import concourse.bacc as bacc
import concourse.mybir as mybir
import concourse.tile as tile
from concourse.ap import AP
from concourse import bass_utils

F32 = mybir.dt.float32
BF16 = mybir.dt.bfloat16
I32 = mybir.dt.int32
ALU = mybir.AluOpType
ACTF = mybir.ActivationFunctionType
AXIS = mybir.AxisListType

BF16NP = mybir.dt.np(mybir.dt.bfloat16)

# ---------------------------------------------------------------- constants
P4 = 4          # primitive cats
NF = 36         # non-functor cats
C = 2596        # total cats
CP = 2688       # padded C (21 * 128)
NT = CP // 128  # 21 c-tiles
D = 64
B = 32          # total sentences
NCORES = 8
BLOC = B // NCORES  # 4 sentences per core
V = 32000
BLK = 80        # per-level block stride in chart tensors
NEGB = -1.0e5   # bias for padded vocab columns

LN2 = 0.6931471805599453
# ln(m) ~= sum_k LNB[k] * m^k on [1,2] (no constant term; max err ~1.1e-4)
LNB = [-4.70528660059376, 11.918040257343248, -12.100112712668084,
       6.47839771749535, -1.7936781454842305, 0.20274855065437586]


class Cfg:
    def __init__(self, n=32, v_loc=4000, n_cores=8):
        self.n = n                      # sentence length
        self.v_loc = v_loc              # vocab shard per core
        self.v_pad = ((v_loc + 511) // 512) * 512
        self.n_cores = n_cores
        self.pairs = 4 * n              # (i, b) pairs on partitions


# ------------------------------------------------------------ functor maps
def lf_block_offsets(op):
    """c = off + {A: 4r+a | B: 32r+(a-4) | C: 36(r-4)+a} per derivation of
    the deterministic functor-id tables. op=0 -> l_functors, 1 -> r_functors."""
    return {
        "A": 4 + 16 * op,            # res<4, arg<4 : c = A + 4*res + arg
        "B": 36 + 1280 * op,         # res<4, arg>=4: c = B + 32*res + (arg-4)
        "C": 164 + 1280 * op,        # res>=4      : c = C0 + 36*(res-4) + arg
    }


def check_functor_tables(l_functors, r_functors):
    for op, tab in ((0, l_functors), (1, r_functors)):
        off = lf_block_offsets(op)
        exp = np.zeros((NF, NF), np.int64)  # [arg, res]
        for res in range(NF):
            for arg in range(NF):
                if res < P4 and arg < P4:
                    exp[arg, res] = off["A"] + 4 * res + arg
                elif res < P4:
                    exp[arg, res] = off["B"] + 32 * res + (arg - 4)
                else:
                    exp[arg, res] = off["C"] + 36 * (res - 4) + arg
        assert np.array_equal(np.asarray(tab, np.int64), exp), (
            f"functor table structure mismatch (op={op})")


# ---------------------------------------------------------------- AP helper
def mk(t, parts, off, dims, base_part=0):
    """Raw AP on tile t: partition range [base_part, base_part+parts),
    free offset `off` (elements), extra free dims [[step, count], ...]."""
    w = t.ap[0][0]
    return AP(t.tensor, t.offset + base_part * w + off, [[w, parts]] + dims)


def dve_ln(nc, out, in_, scr_f, scr_i, p, w):
    """out[0:p, 0:w] = ln(in_[0:p, 0:w]) via exponent extraction + a
    degree-6 polynomial on the mantissa, entirely on the vector engine.
    in_ must be positive fp32.  scr_f fp32 / scr_i int32 scratch tiles
    (>= w cols each).  Max abs err ~1.1e-4."""
    e_i = scr_i[0:p, 0:w]
    m_i = scr_i[0:p, w:2 * w]
    ef = scr_f[0:p, 0:w]
    ib = in_.bitcast(I32)
    # raw exponent - 127
    nc.vector.tensor_scalar(e_i, ib, 23, None, op0=ALU.logical_shift_right)
    nc.vector.tensor_scalar(e_i, e_i, 127, None, op0=ALU.subtract)
    nc.vector.tensor_copy(ef, e_i)          # int -> float convert
    # mantissa in [1, 2): keep mantissa bits, force exponent field to 127
    nc.vector.tensor_scalar(m_i, ib, 0x007FFFFF, 0x3F800000,
                            op0=ALU.bitwise_and, op1=ALU.bitwise_or)
    m = m_i.bitcast(F32)
    # Horner in the (p + b) * m form: p = sum_k LNB[k] m^k
    nc.vector.tensor_scalar_mul(out, m, LNB[5])
    for k in (4, 3, 2, 1, 0):
        nc.vector.scalar_tensor_tensor(out, out, LNB[k], m,
                                       op0=ALU.add, op1=ALU.mult)
    # out = ef * ln2 + poly
    nc.vector.scalar_tensor_tensor(out, ef, LN2, out,
                                   op0=ALU.mult, op1=ALU.add)


# ============================================================ device program
def build_program(cfg: Cfg):
    nc = bacc.Bacc("TRN2", target_bir_lowering=False, debug=False,
                   num_devices=cfg.n_cores)
    d = {
        "ntembT": nc.dram_tensor("ntembT", [65, CP], BF16,
                                 kind="ExternalInput"),
        "vocabW": nc.dram_tensor("vocabW", [65, cfg.v_pad], BF16,
                                 kind="ExternalInput"),
        "wordW": nc.dram_tensor("wordW", [66, cfg.pairs], BF16,
                                kind="ExternalInput"),
        "mlpW": nc.dram_tensor("mlpW", [64, 322], BF16, kind="ExternalInput"),
        "mlpB": nc.dram_tensor("mlpB", [64, 8], F32, kind="ExternalInput"),
        "ruleWb": nc.dram_tensor("ruleWb", [36, 144], F32,
                                 kind="ExternalInput"),
        "smallv": nc.dram_tensor("smallv", [1, 16], F32,
                                 kind="ExternalInput"),
        "sW2b": nc.dram_tensor("sW2b", [65, 2], BF16,
                                kind="ExternalInput"),
        "ident": nc.dram_tensor("ident", [128, 128], F32,
                                kind="ExternalInput"),
        "out": nc.dram_tensor("out_nll", [BLOC, 1], F32,
                              kind="ExternalOutput"),
    }
    with tile.TileContext(nc) as tc:
        _trace(tc, cfg, d)
    nc.compile()
    return nc


def _trace(tc, cfg, d):
    nc = tc.nc
    n, PAIRS, VP = cfg.n, cfg.pairs, cfg.v_pad
    NV = VP // 512                    # 512-col v-tiles per core
    NHALF = (NV + 2) // 3             # ACT chunks of up to 3 v-tiles

    es = contextlib.ExitStack()
    keep = es.enter_context(tc.tile_pool(name="keep", bufs=1))
    dram = es.enter_context(tc.tile_pool(name="dram", bufs=1, space="DRAM"))

    # ---------------- long-lived tensors
    chartA = keep.tile([PAIRS, (n + 1) * BLK], F32)
    chartEv = keep.tile([PAIRS, (n + 1) * BLK], F32)
    chartEo = keep.tile([PAIRS, (n + 1) * BLK], F32)
    WA = keep.tile([PAIRS, 1312], BF16)
    WB = keep.tile([PAIRS, 1312], BF16)
    glR = keep.tile([128, 1296], F32)
    grR = keep.tile([128, 1296], F32)
    mlpB = keep.tile([64, 8], F32)
    smallv = keep.tile([1, 16], F32)
    sumexp_parts = keep.tile([128, NT * NHALF], F32)
    sumexp_loc = keep.tile([128, NT], F32)
    sumexp_g = keep.tile([128, NT], F32)
    lse21 = keep.tile([128, NT], F32)
    sp21 = keep.tile([128, NT], F32)
    adj21 = keep.tile([128, NT], F32)
    ident = keep.tile([128, 128], F32)
    adjT = keep.tile([NT, 128], BF16)
    lnf = keep.tile([128, 24], F32)          # dve_ln scratch
    lni = keep.tile([128, 48], I32)
    s0E = keep.tile([1, 40], F32)
    rsRep = keep.tile([4, 4], F32)
    fin = keep.tile([4, 16], F32)

    nc.sync.dma_start(mlpB[:], d["mlpB"][:])
    nc.sync.dma_start(smallv[:], d["smallv"][:])
    nc.sync.dma_start(ident[:], d["ident"][:])
    nc.gpsimd.memset(chartA[:], 0.0)
    nc.gpsimd.memset(chartEv[:], 0.0)
    nc.gpsimd.memset(chartEo[:], 0.0)

    ph1 = contextlib.ExitStack()
    p1 = ph1.enter_context(tc.tile_pool(name="ph1", bufs=1))
    ntembT = p1.tile([65, CP], BF16)
    vocabW = p1.tile([65, VP], BF16)
    wordW = p1.tile([66, PAIRS], BF16)
    mlpW = p1.tile([64, 322], BF16)
    ruleWb = p1.tile([36, 144], F32)
    rhs_b = p1.tile([66, CP], BF16)
    beta1E = p1.tile([PAIRS, CP], F32)
    ruleflat = p1.tile([1, 36 * 72], F32)
    hA = p1.tile([65, CP], BF16)
    sW2b = p1.tile([65, 2], BF16)
    hB = p1.tile([64, CP], BF16)
    hC = p1.tile([64, CP], BF16)
    y21 = p1.tile([128, NT], F32)
    u21 = p1.tile([128, NT], F32)
    s0col = p1.tile([128, 1], F32)

    nc.sync.dma_start(ntembT[:], d["ntembT"][:])
    nc.sync.dma_start(vocabW[:], d["vocabW"][:])
    nc.sync.dma_start(wordW[:], d["wordW"][:])
    nc.sync.dma_start(mlpW[:], d["mlpW"][:])
    nc.sync.dma_start(sW2b[:], d["sW2b"][:])
    nc.gpsimd.memset(hA[64:65, :], 1.0)
    nc.sync.dma_start(ruleWb[:], d["ruleWb"][:])
    nc.sync.dma_start(rhs_b[0:65, :], ntembT[:])

    # =======================================================================
    # Phase 1: emission partition function (bf16 matmul, exp+accum on scalar)
    # =======================================================================
    with tc.tile_pool(name="psum_e", bufs=2, space="PSUM") as pse, \
         tc.tile_pool(name="scr_e", bufs=2) as scre:
        for ct in range(NT):
            for h in range(NHALF):
                vt0 = h * 3
                nvt = min(3, NV - vt0)
                pt = pse.tile([128, 512 * nvt], F32, tag="pse")
                for vt in range(nvt):
                    nc.tensor.matmul(
                        pt[:, vt * 512:(vt + 1) * 512],
                        ntembT[:, ct * 128:(ct + 1) * 128],
                        vocabW[:, (vt0 + vt) * 512:(vt0 + vt + 1) * 512],
                        start=True, stop=True)
                sce = scre.tile([128, 512 * 3], F32, tag="scre")
                nc.scalar.activation(
                    sce[:, 0:512 * nvt], pt[:], ACTF.Exp,
                    accum_out=sumexp_parts[:, ct * NHALF + h:
                                           ct * NHALF + h + 1])

    if NHALF > 1:
        nc.vector.tensor_reduce(
            sumexp_loc[:],
            mk(sumexp_parts, 128, 0, [[NHALF, NT], [1, NHALF]]),
            axis=AXIS.X, op=ALU.add)
    else:
        nc.vector.tensor_copy(sumexp_loc[:], sumexp_parts[:, 0:NT])

    # AllReduce over cores via DRAM bounce
    cc_in = dram.tile([128, NT], F32)
    cc_out = dram.tile([128, NT], F32)
    nc.sync.dma_start(cc_in[:], sumexp_loc[:])
    nc.gpsimd.collective_compute(
        "AllReduce", ALU.add,
        replica_groups=[list(range(cfg.n_cores))],
        ins=[cc_in[:].opt()], outs=[cc_out[:].opt()])
    nc.sync.dma_start(sumexp_g[:], cc_out[:])

    # =======================================================================
    # Phase 2: split MLP (transposed layout hT [64, CP]), rule tables, root
    # (emitted after emission so the tensor queue drains emission first)
    # =======================================================================
    psm = ph1.enter_context(tc.tile_pool(name="psum_m", bufs=2, space="PSUM"))
    scrs = ph1.enter_context(tc.tile_pool(name="scr_s", bufs=2))
    hp = tc.high_priority

    def dense_relu(dst, col0, rhs, bias_col, res_add=None, func=ACTF.Relu):
        with hp():
            for c0 in range(0, CP, 512):
                c1 = min(c0 + 512, CP)
                pm = psm.tile([128, 512], F32, tag="psm")
                nc.tensor.matmul(pm[0:64, 0:c1 - c0],
                                 mlpW[:, col0:col0 + 64],
                                 rhs[0:64, c0:c1],
                                 start=True, stop=True)
                nc.scalar.activation(
                    dst[0:64, c0:c1], pm[0:64, 0:c1 - c0], func,
                    bias=mlpB[:, bias_col:bias_col + 1])
                if res_add is not None:
                    nc.vector.tensor_tensor(
                        dst[0:64, c0:c1], dst[0:64, c0:c1],
                        res_add[0:64, c0:c1], op=ALU.add)

    dense_relu(hA, 0, ntembT, 0, func=ACTF.Identity)   # h1 (linear)
    dense_relu(hB, 64, hA, 1)                           # t = relu(h1 W + b)
    dense_relu(hC, 128, hB, 2, res_add=hA)              # h2
    dense_relu(hB, 192, hC, 3)                          # t2
    dense_relu(hA, 256, hB, 4, res_add=hC)              # h3 (rows 0:64)

    # split head, transposed: per c-tile psum [128, 2] incl sb2 bias row;
    # y = s0 - s1
    with hp():
        for ct in range(NT):
            ps = psm.tile([128, 512], F32, tag="psm")
            nc.tensor.matmul(ps[0:128, 0:2], hA[:, ct * 128:(ct + 1) * 128],
                             sW2b[:], start=True, stop=True)
            y2t = scrs.tile([128, 2], F32, tag="y2t")
            nc.vector.tensor_copy(y2t[:], ps[0:128, 0:2])
            nc.vector.tensor_tensor(
                y21[:, ct:ct + 1], y2t[:, 0:1], y2t[:, 1:2], op=ALU.subtract)

    # softplus(y) = max(y,0) + ln(1 + exp(-|y|)) with DVE ln
    with hp():
        nc.vector.tensor_scalar(u21[:].bitcast(I32), y21[:].bitcast(I32),
                                0x7FFFFFFF, None, op0=ALU.bitwise_and)
        nc.scalar.activation(u21[:], u21[:], ACTF.Exp, scale=-1.0)
        nc.vector.tensor_scalar_add(u21[:], u21[:], 1.0)
        dve_ln(nc, u21[:], u21[:], lnf, lni, 128, NT)   # ln(1+e^-|y|)
        nc.vector.tensor_scalar_max(sp21[:], y21[:], 0.0)
        nc.vector.tensor_tensor(sp21[:], sp21[:], u21[:], op=ALU.add)

        # split0 = y - softplus(y); s0E = exp(split0) for cats 0..35 as a row
        nc.vector.tensor_tensor(s0col[:], y21[:, 0:1], sp21[:, 0:1],
                                op=ALU.subtract)
        nc.scalar.activation(s0col[:], s0col[:], ACTF.Exp)
        s0_d = dram.tile([NF, 1], F32)
        nc.sync.dma_start(s0_d[:], s0col[0:NF, :])
        nc.sync.dma_start(
            AP(s0E.tensor, s0E.offset, [[s0E.ap[0][0], 1], [1, NF]]),
            AP(s0_d.tensor, s0_d.offset, [[s0_d.ap[0][0], 1], [1, NF]]))

    # rule tables: softmax over 72 per res row
    hp_rules = hp()
    hp_rules.__enter__()
    rsum = keep.tile([36, 72], F32)
    rmax = keep.tile([36, 2], F32)
    rsumexp = keep.tile([36, 2], F32)
    nc.vector.tensor_tensor(rsum[:], ruleWb[:, 0:72], ruleWb[:, 72:144],
                            op=ALU.add)
    nc.vector.tensor_reduce(rmax[:, 0:1], rsum[:], axis=AXIS.X, op=ALU.max)
    nc.vector.tensor_scalar_mul(rmax[:, 1:2], rmax[:, 0:1], -1.0)
    nc.scalar.activation(rsum[:], rsum[:], ACTF.Exp, bias=rmax[:, 1:2],
                         accum_out=rsumexp[:, 0:1])
    nc.vector.reciprocal(rsumexp[:, 1:2], rsumexp[:, 0:1])
    nc.vector.tensor_scalar_mul(rsum[:], rsum[:], rsumexp[:, 1:2])

    # flatten ruleEn to [1, 2592] via DRAM, then G-flats replicated
    rule_d = dram.tile([36, 72], F32)
    nc.sync.dma_start(rule_d[:], rsum[:])
    nc.sync.dma_start(
        AP(ruleflat.tensor, ruleflat.offset,
           [[ruleflat.ap[0][0], 1], [1, 36 * 72]]),
        rule_d[:])
    g_d = dram.tile([2, 1296], F32)
    gtmp = keep.tile([1, 1296], F32)
    for row, off in ((0, 0), (1, 36)):   # 0: Gl (larg), 1: Gr (rarg)
        nc.vector.tensor_tensor(
            gtmp[:],
            mk(ruleflat, 1, off, [[72, 36], [1, 36]]),
            mk(s0E, 1, 0, [[1, 36], [0, 36]]),
            op=ALU.mult)
        nc.sync.dma_start(g_d[row:row + 1, :], gtmp[:])
    for dstt, row in ((glR, 0), (grR, 1)):
        nc.sync.dma_start(
            dstt[:],
            AP(g_d.tensor, g_d.offset + row * g_d.ap[0][0],
               [[0, 128], [1, 1296]]))

    # root: rsEn = softmax(root_W[0,0:4] + root_b[0:4]) replicated to 4 parts
    rs4 = keep.tile([1, 8], F32)
    rsE = keep.tile([1, 8], F32)
    nc.vector.tensor_tensor(rs4[:, 0:4], smallv[:, 2:6], smallv[:, 6:10],
                            op=ALU.add)
    nc.vector.tensor_reduce(rs4[:, 4:5], rs4[:, 0:4], axis=AXIS.X, op=ALU.max)
    nc.vector.tensor_scalar_mul(rs4[:, 5:6], rs4[:, 4:5], -1.0)
    nc.scalar.activation(rsE[:, 0:4], rs4[:, 0:4], ACTF.Exp,
                         bias=rs4[:, 5:6], accum_out=rsE[:, 4:5])
    nc.vector.reciprocal(rsE[:, 5:6], rsE[:, 4:5])
    nc.vector.tensor_scalar_mul(rsE[:, 0:4], rsE[:, 0:4], rsE[:, 5:6])
    rs_d = dram.tile([1, 4], F32)
    nc.sync.dma_start(rs_d[:], rsE[:, 0:4])
    nc.sync.dma_start(rsRep[:],
                      AP(rs_d.tensor, rs_d.offset, [[0, 4], [1, 4]]))
    hp_rules.__exit__(None, None, None)

    # =======================================================================
    # Phase 3: adj -> beta1 logits -> exp tables WA/WB (M1 == 0: beta1 log
    # values are <= 0 because dist is a log-softmax and split1 <= 0)
    # =======================================================================
    # adj = -softplus(y) - lse, in [128, NT] layout; PE-transpose to a row
    with hp():
        dve_ln(nc, lse21[:], sumexp_g[:], lnf, lni, 128, NT)
        nc.vector.scalar_tensor_tensor(adj21[:], sp21[:], -1.0, lse21[:],
                                       op0=ALU.mult, op1=ALU.subtract)
        with tc.tile_pool(name="psum_t", bufs=1, space="PSUM") as pst:
            pT = pst.tile([NT, 128], F32)
            nc.tensor.transpose(pT[:], adj21[:], ident[:])
            nc.vector.tensor_copy(adjT[:], pT[:])   # fp32 psum -> bf16 sbuf
        nc.sync.dma_start(
            AP(rhs_b.tensor, rhs_b.offset + 65 * rhs_b.ap[0][0],
               [[rhs_b.ap[0][0], 1], [128, NT], [1, 128]]),
            adjT[:])

    with tc.tile_pool(name="psum_b", bufs=1, space="PSUM") as psb:
        pb = psb.tile([PAIRS, CP], F32)
        for c0 in range(0, CP, 512):
            c1 = min(c0 + 512, CP)
            nc.tensor.matmul(pb[:, c0:c1], wordW[:], rhs_b[:, c0:c1],
                             start=True, stop=True)
        nc.scalar.activation(beta1E[:], pb[:], ACTF.Exp)

    # W tables [PAIRS, 1296] bf16: WB = gather_lf(beta1E)*GrE, WA = rf/GlE
    for W, op_id, gR in ((WB, 0, grR), (WA, 1, glR)):
        off = lf_block_offsets(op_id)
        blocks = [
            (0, [[36, 4], [1, 4]], off["A"], [[4, 4], [1, 4]]),
            (4, [[36, 4], [1, 32]], off["B"], [[32, 4], [1, 32]]),
            (144, [[1, 1152]], off["C"], [[1, 1152]]),
        ]
        for (oo, od, io, idm) in blocks:
            nc.vector.scalar_tensor_tensor(
                mk(W, PAIRS, oo, od),
                mk(beta1E, PAIRS, io, idm),
                1.0,
                mk(gR, PAIRS, oo, od),
                op0=ALU.mult, op1=ALU.mult)

    # chart block L=1 from beta1E (scale col stays 0 from the memset)
    nc.vector.tensor_copy(mk(chartA, PAIRS, BLK, [[1, 36]]), beta1E[:, 0:NF])
    nc.vector.tensor_tensor(mk(chartA, PAIRS, BLK + 40, [[4, 4], [1, 4]]),
                            mk(beta1E, PAIRS, 20, [[4, 4], [1, 4]]),
                            mk(glR, PAIRS, 0, [[36, 4], [1, 4]]),
                            op=ALU.mult)
    nc.vector.tensor_tensor(mk(chartA, PAIRS, BLK + 56, [[4, 4], [1, 4]]),
                            mk(beta1E, PAIRS, 4, [[4, 4], [1, 4]]),
                            mk(grR, PAIRS, 0, [[36, 4], [1, 4]]),
                            op=ALU.mult)
    # chartEnd block m lives at col (n-m)*BLK (reversed layout; makes all
    # k-strided reads positive-step). Block 1: end j = i+1 -> row pair.
    nc.sync.dma_start(mk(chartEo, PAIRS, (n - 1) * BLK, [[1, BLK]]),
                      mk(chartA, PAIRS, BLK, [[1, BLK]]))

    ph1.close()  # free ph1 tensors before the CKY working set

    es2 = contextlib.ExitStack()
    stage_pool = es2.enter_context(tc.tile_pool(name="stage", bufs=2))
    wash_pool = es2.enter_context(tc.tile_pool(name="wash", bufs=1))
    scr = es2.enter_context(tc.tile_pool(name="cky", bufs=2))
    scr1 = es2.enter_context(tc.tile_pool(name="cky1", bufs=1))

    # =======================================================================
    # Phase 4: CKY in scaled-exp space
    # chartA[pair, L*BLK+.]: 0:36 values | 36 scale | 40:56 FA | 56:72 FB
    # chartE[(j-1)*4+b, ...] same, indexed by span end j.
    # =======================================================================
    for L in range(2, n + 1):
        S = n - L + 1
        PS = 4 * S
        NI = L - 2

        stageE = stage_pool.tile([128, n * BLK], F32, tag="st")
        # critical part first on the sync queue: block L-1 rows [4 .. 4+PS]
        nc.sync.dma_start(
            mk(stageE, PS, (n - L + 1) * BLK, [[1, BLK]]),
            mk(chartA, PS, (L - 1) * BLK, [[1, BLK]], base_part=4))
        # prefetchable part: blocks 1..L-2 on the scalar queue, split by the
        # parity of the writing iteration so a fresh writeback never blocks it
        for par, chE in ((0, chartEv), (1, chartEo)):
            ms = [m for m in range(1, L - 1) if m % 2 == par]
            if not ms:
                continue
            mmax = max(ms)
            nc.gpsimd.dma_start(
                mk(stageE, PS, (n - mmax) * BLK,
                   [[2 * BLK, len(ms)], [1, BLK]]),
                mk(chE, PS, (n - mmax) * BLK,
                   [[2 * BLK, len(ms)], [1, BLK]], base_part=4 * (L - 1)))

        wash = wash_pool.tile([128, 1312], BF16, tag="wa")
        nc.gpsimd.dma_start(
            mk(wash, PS, 0, [[1, 1296]]),
            mk(WA, PS, 0, [[1, 1296]], base_part=4 * (L - 1)))

        # ---- scales: sAsm = [sB | sA | sI(k=1..L-1)]
        sAsm = scr.tile([128, n + 8], F32, tag="sasm")
        nc.vector.tensor_copy(
            sAsm[0:PS, 0:1],
            mk(stageE, PS, (n - L + 1) * BLK + 36, [[1, 1]]))
        nc.vector.tensor_copy(
            sAsm[0:PS, 1:2],
            mk(chartA, PS, (L - 1) * BLK + 36, [[1, 1]]))
        nc.vector.tensor_tensor(
            sAsm[0:PS, 2:L + 1],
            mk(chartA, PS, BLK + 36, [[BLK, L - 1]]),
            mk(stageE, PS, (n - L + 1) * BLK + 36, [[BLK, L - 1]]),
            op=ALU.add)
        mstar = scr.tile([128, 2], F32, tag="mstar")
        nc.vector.tensor_reduce(mstar[0:PS, 0:1], sAsm[0:PS, 0:L + 1],
                                axis=AXIS.X, op=ALU.max)
        nc.vector.tensor_scalar_mul(mstar[0:PS, 1:2], mstar[0:PS, 0:1], -1.0)
        eAll = scr.tile([128, n + 8], F32, tag="eall")
        nc.scalar.activation(eAll[0:PS, 0:L + 1], sAsm[0:PS, 0:L + 1],
                             ACTF.Exp, bias=mstar[0:PS, 1:2])

        # ---- edge products in bf16 (DVE 2x): prodAB [PS, 2592]
        ea36 = scr.tile([128, 72], BF16, tag="ea36")
        nc.vector.tensor_scalar_mul(
            ea36[0:PS, 0:36],
            mk(chartA, PS, (L - 1) * BLK, [[1, 36]]), eAll[0:PS, 1:2])
        nc.vector.tensor_scalar_mul(
            ea36[0:PS, 36:72],
            mk(stageE, PS, (n - L + 1) * BLK, [[1, 36]]), eAll[0:PS, 0:1])
        prodAB = scr1.tile([128, 2592], BF16, tag="prod")
        nc.vector.tensor_tensor(
            mk(prodAB, PS, 0, [[1296, 2], [36, 36], [1, 36]]),
            mk(wash, PS, 0, [[1312, 2], [36, 36], [1, 36]]),
            mk(ea36, PS, 0, [[36, 2], [0, 36], [1, 36]]),
            op=ALU.mult)
        redT = scr.tile([128, 1296], BF16, tag="redt")
        nc.vector.tensor_tensor(          # fold 36 -> 18 (bf16, 4x)
            mk(redT, PS, 0, [[18, 72], [1, 18]]),
            mk(prodAB, PS, 0, [[36, 72], [1, 18]]),
            mk(prodAB, PS, 18, [[36, 72], [1, 18]]), op=ALU.add)
        red9 = scr.tile([128, 648], BF16, tag="red9")
        nc.vector.tensor_tensor(          # fold 18 -> 9 (bf16, 4x)
            mk(red9, PS, 0, [[9, 72], [1, 9]]),
            mk(redT, PS, 0, [[18, 72], [1, 9]]),
            mk(redT, PS, 9, [[18, 72], [1, 9]]), op=ALU.add)
        red72 = scr.tile([128, 72], BF16, tag="red")
        with nc.allow_low_precision("bf16 CKY group sums, tol 2e-2"):
            nc.vector.tensor_reduce(red72[0:PS, :],
                                    mk(red9, PS, 0, [[9, 72], [1, 9]]),
                                    axis=AXIS.X, op=ALU.add)
        total36 = scr.tile([128, 40], F32, tag="tot")
        tdst = (total36[0:PS, 0:36] if L % 2 == 0
                else mk(chartA, PS, L * BLK, [[1, 36]]))
        nc.vector.tensor_tensor(tdst, red72[0:PS, 0:36],
                                red72[0:PS, 36:72], op=ALU.add)

        # ---- interior terms (res<4), batched over k
        if NI > 0:
            tI = scr1.tile([128, 2 * max(n - 2, 1) * 16], F32, tag="ti")
            nc.vector.tensor_tensor(   # IA: chart[k][i] args x stage FA(L-k)
                mk(tI, PS, 0, [[2 * NI * 4, 4], [4, NI], [1, 4]]),
                mk(chartA, PS, BLK, [[0, 4], [BLK, NI], [1, 4]]),
                mk(stageE, PS, (n - L + 1) * BLK + 40,
                   [[4, 4], [BLK, NI], [1, 4]]),
                op=ALU.mult)
            nc.vector.tensor_tensor(   # IB: stage args(L-k) x chart[k] FB
                mk(tI, PS, NI * 4, [[2 * NI * 4, 4], [4, NI], [1, 4]]),
                mk(stageE, PS, (n - L + 2) * BLK, [[0, 4], [BLK, NI], [1, 4]]),
                mk(chartA, PS, 2 * BLK + 56, [[4, 4], [BLK, NI], [1, 4]]),
                op=ALU.mult)
            for half in range(2):      # x eI (k scales), in place
                nc.vector.tensor_tensor(
                    mk(tI, PS, half * NI * 4,
                       [[2 * NI * 4, 4], [4, NI], [1, 4]]),
                    mk(tI, PS, half * NI * 4,
                       [[2 * NI * 4, 4], [4, NI], [1, 4]]),
                    mk(eAll, PS, 2 + half, [[0, 4], [1, NI], [0, 4]]),
                    op=ALU.mult)
            nc.vector.tensor_reduce(   # sum over (side*k, arg) -> [PS, 4]
                total36[0:PS, 36:40],
                mk(tI, PS, 0,
                   [[2 * NI * 4, 4], [4, 2 * NI], [1, 4]]),
                axis=AXIS.XY, op=ALU.add)
            idst = (total36[0:PS, 0:4] if L % 2 == 0
                    else mk(chartA, PS, L * BLK, [[1, 4]]))
            nc.vector.tensor_tensor(idst, idst,
                                    total36[0:PS, 36:40], op=ALU.add)

        # ---- rescale by a power of two (integer bit ops; no Ln, no recip).
        # Values drift at most ~e^-40/level, so renormalizing every other
        # level keeps everything in fp32 range at half the bookkeeping.
        if L % 2 == 0:
            mval = scr.tile([128, 2], F32, tag="mval")
            mvi = scr.tile([128, 4], I32, tag="mvi")
            nc.vector.tensor_reduce(mval[0:PS, 0:1], total36[0:PS, 0:36],
                                    axis=AXIS.X, op=ALU.max)
            nc.vector.tensor_scalar(          # t = raw_exponent - 127
                mvi[0:PS, 0:1], mval[0:PS, 0:1].bitcast(I32), 23, None,
                op0=ALU.logical_shift_right)
            nc.vector.tensor_scalar(
                mvi[0:PS, 0:1], mvi[0:PS, 0:1], 127, None, op0=ALU.subtract)
            nc.vector.tensor_copy(mval[0:PS, 1:2], mvi[0:PS, 0:1])  # float(t)
            nc.vector.scalar_tensor_tensor(   # scale_L = t*ln2 + mstar
                mk(chartA, PS, L * BLK + 36, [[1, 1]]),
                mval[0:PS, 1:2], LN2, mstar[0:PS, 0:1],
                op0=ALU.mult, op1=ALU.add)
            nc.vector.tensor_scalar(          # recip bits = (127 - t) << 23
                mvi[0:PS, 1:2], mvi[0:PS, 0:1], -1, 127,
                op0=ALU.mult, op1=ALU.add)
            nc.vector.tensor_scalar(
                mvi[0:PS, 1:2], mvi[0:PS, 1:2], 23, None,
                op0=ALU.logical_shift_left)
            nc.vector.tensor_scalar_mul(
                mk(chartA, PS, L * BLK, [[1, 36]]),
                total36[0:PS, 0:36], mvi[0:PS, 1:2].bitcast(F32))
        else:
            nc.vector.tensor_copy(mk(chartA, PS, L * BLK + 36, [[1, 1]]),
                                  mstar[0:PS, 0:1])
        nc.vector.tensor_tensor(
            mk(chartA, PS, L * BLK + 40, [[4, 4], [1, 4]]),
            mk(chartA, PS, L * BLK + 20, [[4, 4], [1, 4]]),
            mk(glR, PS, 0, [[36, 4], [1, 4]]), op=ALU.mult)
        nc.vector.tensor_tensor(
            mk(chartA, PS, L * BLK + 56, [[4, 4], [1, 4]]),
            mk(chartA, PS, L * BLK + 4, [[4, 4], [1, 4]]),
            mk(grR, PS, 0, [[36, 4], [1, 4]]), op=ALU.mult)
        if L < n:   # chartEnd block L at rows (i+L-1)*4+b, col (n-L)*BLK
            nc.gpsimd.dma_start(
                mk(chartEo if L % 2 else chartEv, PS, (n - L) * BLK,
                   [[1, BLK]], base_part=4 * (L - 1)),
                mk(chartA, PS, L * BLK, [[1, BLK]]))

    # =======================================================================
    # Phase 5: root -> nll per sentence
    # =======================================================================
    nc.vector.tensor_tensor(fin[:, 0:4],
                            mk(chartA, 4, n * BLK, [[1, 4]]),
                            rsRep[:], op=ALU.mult)
    nc.vector.tensor_reduce(fin[:, 4:5], fin[:, 0:4], axis=AXIS.X, op=ALU.add)
    dve_ln(nc, fin[:, 5:6], fin[:, 4:5], lnf, lni, 4, 1)
    nc.vector.scalar_tensor_tensor(
        fin[:, 6:7], fin[:, 5:6], -1.0,
        mk(chartA, 4, n * BLK + 36, [[1, 1]]),
        op0=ALU.mult, op1=ALU.subtract)
    nc.sync.dma_start(d["out"][:], fin[:, 6:7])
    es2.close()
    es.close()


# ============================================================== host wrapper
_PROG_CACHE = {}


def _get_program(cfg: Cfg):
    key = (cfg.n, cfg.v_loc, cfg.n_cores)
    if key not in _PROG_CACHE:
        _PROG_CACHE[key] = build_program(cfg)
    return _PROG_CACHE[key]


def make_inmaps(cfg: Cfg, inputs):
    """Host-side shard/pack of FULL inputs -> per-core DRAM input dicts."""
    x = np.asarray(inputs["x"])
    check_functor_tables(np.asarray(inputs["l_functors"]),
                         np.asarray(inputs["r_functors"]))
    nt_emb = np.asarray(inputs["nt_emb"], np.float32)          # [C, D]
    vocab_W = np.asarray(inputs["vocab_W"], np.float32)        # [D, V]
    vocab_b = np.asarray(inputs["vocab_b"], np.float32)        # [V]

    ntembT = np.zeros((65, CP), np.float32)
    ntembT[0:64, 0:C] = nt_emb.T
    ntembT[64, :] = 1.0
    ntembT = ntembT.astype(BF16NP)

    mlpW = np.zeros((64, 322), np.float32)
    for j, k in enumerate(("sW1", "r1W1", "r1W2", "r2W1", "r2W2")):
        mlpW[:, j * 64:(j + 1) * 64] = np.asarray(inputs[k], np.float32)
    mlpW[:, 320:322] = np.asarray(inputs["sW2"], np.float32)
    mlpW = mlpW.astype(BF16NP)

    mlpB = np.zeros((64, 8), np.float32)
    for j, k in enumerate(("sb1", "r1b1", "r1b2", "r2b1", "r2b2")):
        mlpB[:, j] = np.asarray(inputs[k], np.float32)

    ruleWb = np.zeros((36, 144), np.float32)
    ruleWb[:, 0:72] = np.asarray(inputs["rule_W"], np.float32)
    ruleWb[:, 72:144] = np.tile(
        np.asarray(inputs["rule_b"], np.float32)[None, :], (36, 1))

    smallv = np.zeros((1, 16), np.float32)
    smallv[0, 0:2] = np.asarray(inputs["sb2"], np.float32)
    smallv[0, 2:6] = np.asarray(inputs["root_W"], np.float32)[0, 0:4]
    smallv[0, 6:10] = np.asarray(inputs["root_b"], np.float32)[0:4]

    ident_np = np.eye(128, dtype=np.float32)
    sW2b = np.zeros((65, 2), np.float32)
    sW2b[0:64, :] = np.asarray(inputs["sW2"], np.float32)
    sW2b[64, :] = np.asarray(inputs["sb2"], np.float32)
    sW2b = sW2b.astype(BF16NP)

    vs = cfg.v_loc
    in_maps = []
    for core in range(cfg.n_cores):
        vocabW = np.zeros((65, cfg.v_pad), np.float32)
        vocabW[64, :] = NEGB
        vocabW[0:64, 0:vs] = vocab_W[:, core * vs:(core + 1) * vs]
        vocabW[64, 0:vs] = vocab_b[core * vs:(core + 1) * vs]

        words = x[core * BLOC:(core + 1) * BLOC, 0:cfg.n]   # [BLOC, n]
        wid = words.T.reshape(-1)                           # pair = i*4 + b
        wordW = np.zeros((66, cfg.pairs), np.float32)
        wordW[0:64, :] = vocab_W[:, wid]
        wordW[64, :] = vocab_b[wid]
        wordW[65, :] = 1.0

        in_maps.append({
            "ntembT": ntembT, "vocabW": vocabW.astype(BF16NP),
            "wordW": wordW.astype(BF16NP),
            "mlpW": mlpW, "mlpB": mlpB, "ruleWb": ruleWb, "smallv": smallv,
            "sW2b": sW2b, "ident": ident_np,
        })
    return in_maps


def kernel(**inputs) -> np.ndarray:
    cfg = Cfg(n=32, v_loc=V // NCORES, n_cores=NCORES)
    nc = _get_program(cfg)
    in_maps = make_inmaps(cfg, inputs)
    res = bass_utils.run_bass_kernel_spmd(
        nc, in_maps, core_ids=list(range(cfg.n_cores)))
    out = np.concatenate([r["out_nll"].reshape(-1) for r in res.results])
    return out.astype(np.float32)


if __name__ == "__main__":
    from reference import setup_inputs, reference
    inputs = {k: np.asarray(v) for k, v in setup_inputs().items()}
    got = kernel(**inputs)
    exp = np.asarray(reference(**inputs))
    rel = np.max(np.abs(got - exp) / np.maximum(np.abs(exp), 1e-6))
    print("expected:", exp[:8])
    print("got     :", got[:8])
    print("Relative error:", rel)
